# revision 18
# baseline (speedup 1.0000x reference)
"""Trainium2 Bass kernel for the D4 codebook (VQ) problem.

Instead of a 256-way argmax, exploits the structure of the codebook: it is
exactly the set of points in the coset (Z+1/2)^4 with even coordinate sum and
norm^2 <= 9.  The nearest codeword is found by scoring 7 magnitude-pattern
types against the sorted |x| values, with a parity-forced sign flip on the
smallest coordinate.  Everything is elementwise f32, no matmul, no gather.

Self-contained: hardcodes shapes (X: [1048576, 4] f32), shards row-blocks
across 8 NeuronCores.
"""

import numpy as np

N_VECS = 1048576
N_CORES = 8
N_PER_CORE = N_VECS // N_CORES  # 131072
P = 128


def _fix_waits(nc):
    """Hoist per-instruction sem waits beyond walrus codegen capacity onto
    standalone EventSemaphore ops inserted just before the instruction.
    Capacities (empirical, this walrus build): EventSemaphore 2, Drain 0,
    2-tensor-input TensorScalarPtr (scalar_tensor_tensor / scan) 0, rest 1."""
    import concourse.mybir as mybir

    def capacity(inst):
        tn = type(inst).__name__
        if tn == "InstEventSemaphore":
            return 2
        if tn == "InstDrain":
            return 0
        if tn == "InstTensorScalarPtr":
            if len(inst.ins or []) >= 2:
                return 0
            return 1
        return 1

    n_fixed = 0
    for f in nc.m.functions:
        for bb in f.blocks:
            out = []
            changed = False
            for inst in bb.instructions:
                si = inst.sync_info
                waits = list(si.on_wait) if si is not None and si.on_wait else []
                cap = capacity(inst)
                if len(waits) > cap:
                    hoist = waits[: len(waits) - cap] if cap else waits
                    keep = waits[len(waits) - cap:] if cap else []
                    for k in range(0, len(hoist), 2):
                        grp = hoist[k:k + 2]
                        es = mybir.InstEventSemaphore(
                            name=f"{inst.name}_hw{k}", ins=[], outs=[]
                        )
                        es.engine = inst.engine
                        es.sync_info = mybir.SyncInfo(on_wait=grp, on_update=[])
                        out.append(es)
                        n_fixed += len(grp)
                    inst.sync_info = mybir.SyncInfo(
                        on_wait=keep,
                        on_update=list(si.on_update) if si.on_update else [],
                    )
                    changed = True
                out.append(inst)
            if changed:
                bb.instructions[:] = out
    return n_fixed


def build_nc(nf, fix_waits=True, reps=1, mode="va"):
    """Build the Bass program for one core processing P*nf vectors.

    mode: "va" (default) = DVE + ACT for affines, no GPSIMD (cross-engine
    sync with Pool measured far too costly); "allv" = everything on DVE;
    "mixed" = spread across DVE/GPSIMD/ACT.
    """
    import concourse.bass as bass
    import concourse.mybir as mybir
    import concourse.tile as tile

    dt = mybir.dt
    A = mybir.AluOpType
    Act = mybir.ActivationFunctionType

    n = P * nf
    fd = nf * 4

    nc = bass.Bass()
    x_h = nc.dram_tensor("x", [n, 4], dt.float32, kind="ExternalInput")
    vals_h = nc.dram_tensor("vals", [n, 4], dt.float32, kind="ExternalOutput")
    idx_h = nc.dram_tensor("idx", [n], dt.uint8, kind="ExternalOutput")

    x_d = x_h[:, :].rearrange("(p n) d -> p (n d)", p=P)
    vals_d = vals_h[:, :].rearrange("(p n) d -> p (n d)", p=P)
    idx_d = idx_h[:].rearrange("(p n) -> p n", p=P)

    with tile.TileContext(nc) as tc:
        with (
            tc.tile_pool(name="wide", bufs=1) as wp,
            tc.tile_pool(name="pers", bufs=1) as pp,
            tc.tile_pool(name="scr", bufs=20) as sp,
        ):
            V = nc.vector
            G = nc.gpsimd if mode == "mixed" else nc.vector
            S = nc.scalar
            use_act = mode in ("mixed", "va")
            g_is_pool = mode == "mixed"

            def AFF(out, in_, scale=1.0, bias=0.0):
                """affine: out = in_*scale + bias"""
                if use_act:
                    S.activation(out, in_, Act.Copy, bias=bias, scale=scale)
                else:
                    V.tensor_scalar(out, in_, scale, bias, A.mult, A.add)

            def G_CMP(out, a, b, op):
                """tensor-tensor compare on G (pool lacks TT compares)"""
                if g_is_pool:
                    G.tensor_tensor(out, a, b, A.subtract)
                    G.tensor_scalar(out, out, 0.0, None, op)
                else:
                    G.tensor_tensor(out, a, b, op)

            def nt(pool, tag):
                return pool.tile([P, nf], dt.float32, tag=tag, name=tag)

            def ntb(pool, tag):
                return pool.tile([P, nf], dt.bfloat16, tag=tag + "b", name=tag)

            def sc():
                return nt(sp, "s")

            def scb():
                return ntb(sp, "s")

            for _rep in range(reps):
                # ---- load ----
                xt = wp.tile([P, fd], dt.float32, tag="xt", name="xt")
                nc.sync.dma_start(xt[:, :], x_d)
                xv = [xt[:, i::4] for i in range(4)]

                # ---- wide: y = |x|, cc = sign in {-1,+1} ----
                y = wp.tile([P, fd], dt.float32, tag="y", name="y")
                S.activation(y[:, :], xt[:, :], Act.Abs)
                yv = [y[:, i::4] for i in range(4)]

                cc = wp.tile([P, fd], dt.bfloat16, tag="cc", name="cc")
                G.tensor_scalar(cc[:, :], xt[:, :], 0.0, 2.0, A.is_ge, A.mult)
                AFF(cc[:, :], cc[:, :], 1.0, -1.0)
                ccv = [cc[:, i::4] for i in range(4)]

                # ---- q: parity of #negatives via product sign ----
                p01 = sc(); G.tensor_tensor(p01[:, :], xv[0], xv[1], A.mult)
                p23 = sc(); G.tensor_tensor(p23[:, :], xv[2], xv[3], A.mult)
                G.tensor_tensor(p01[:, :], p01[:, :], p23[:, :], A.mult)
                q = nt(pp, "q"); G.tensor_scalar(q[:, :], p01[:, :], 0.0, None, A.is_lt)

                # ---- sort network: s1 <= . <= s3 <= s4 over |x| ----
                L1 = sc(); V.tensor_tensor(L1[:, :], yv[0], yv[1], A.min)
                H1 = sc(); V.tensor_tensor(H1[:, :], yv[0], yv[1], A.max)
                L2 = sc(); V.tensor_tensor(L2[:, :], yv[2], yv[3], A.min)
                H2 = sc(); V.tensor_tensor(H2[:, :], yv[2], yv[3], A.max)
                s1 = sc(); V.tensor_tensor(s1[:, :], L1[:, :], L2[:, :], A.min)
                mm = sc(); V.tensor_tensor(mm[:, :], L1[:, :], L2[:, :], A.max)
                mh = sc(); V.tensor_tensor(mh[:, :], H1[:, :], H2[:, :], A.min)
                s4 = sc(); V.tensor_tensor(s4[:, :], H1[:, :], H2[:, :], A.max)
                s3 = sc(); V.tensor_tensor(s3[:, :], mm[:, :], mh[:, :], A.max)

                # T = sum |x|
                Ta = sc(); G.tensor_tensor(Ta[:, :], L1[:, :], H1[:, :], A.add)
                Tb = sc(); G.tensor_tensor(Tb[:, :], L2[:, :], H2[:, :], A.add)
                T = sc(); G.tensor_tensor(T[:, :], Ta[:, :], Tb[:, :], A.add)

                # ---- 7 type scores ----
                w12 = sc(); AFF(w12[:, :], s1[:, :], 2.0)
                e2 = sc(); G.tensor_tensor(e2[:, :], w12[:, :], q[:, :], A.mult)
                f2 = sc(); G.tensor_tensor(f2[:, :], w12[:, :], e2[:, :], A.subtract)
                e23 = sc(); AFF(e23[:, :], e2[:, :], 3.0)

                t0 = sc(); V.scalar_tensor_tensor(t0[:, :], T[:, :], -1.0, e2[:, :], A.add, A.subtract)
                t1 = sc(); V.scalar_tensor_tensor(t1[:, :], s4[:, :], 2.0, T[:, :], A.mult, A.add)  # U
                t2 = sc(); V.scalar_tensor_tensor(t2[:, :], s3[:, :], 2.0, t1[:, :], A.mult, A.add)  # U+2s3
                V.scalar_tensor_tensor(t1[:, :], t1[:, :], -3.0, f2[:, :], A.add, A.subtract)
                V.scalar_tensor_tensor(t2[:, :], t2[:, :], -5.0, e2[:, :], A.add, A.subtract)
                t5 = sc(); V.scalar_tensor_tensor(t5[:, :], s4[:, :], 4.0, T[:, :], A.mult, A.add)  # W
                t6 = sc(); V.scalar_tensor_tensor(t6[:, :], s3[:, :], 2.0, t5[:, :], A.mult, A.add)  # W+2s3
                V.scalar_tensor_tensor(t5[:, :], t5[:, :], -7.0, e2[:, :], A.add, A.subtract)
                V.scalar_tensor_tensor(t6[:, :], t6[:, :], -9.0, f2[:, :], A.add, A.subtract)
                t3 = sc(); V.scalar_tensor_tensor(t3[:, :], T[:, :], 3.0, w12[:, :], A.mult, A.subtract)  # 3T-2s1
                V.scalar_tensor_tensor(t3[:, :], t3[:, :], -7.0, f2[:, :], A.add, A.subtract)
                t4 = sc(); AFF(t4[:, :], T[:, :], 3.0)
                V.scalar_tensor_tensor(t4[:, :], t4[:, :], -9.0, e23[:, :], A.add, A.subtract)
                ts_ = [t0, t1, t2, t3, t4, t5, t6]

                # ---- winner masks via comparison tree ----
                # preference order (ties go left): t0 > t4 > t2 > t1 > t3 > t5 > t6
                ca = scb(); V.tensor_tensor(ca[:, :], t0[:, :], t4[:, :], A.is_ge)
                cb = scb(); V.tensor_tensor(cb[:, :], t2[:, :], t1[:, :], A.is_ge)
                cf_ = scb(); V.tensor_tensor(cf_[:, :], t3[:, :], t5[:, :], A.is_ge)
                w1 = sc(); V.tensor_tensor(w1[:, :], t0[:, :], t4[:, :], A.max)
                w2 = sc(); V.tensor_tensor(w2[:, :], t2[:, :], t1[:, :], A.max)
                w3 = sc(); V.tensor_tensor(w3[:, :], t3[:, :], t5[:, :], A.max)
                cd = scb(); V.tensor_tensor(cd[:, :], w1[:, :], w2[:, :], A.is_ge)
                V.tensor_tensor(w1[:, :], w1[:, :], w2[:, :], A.max)  # w12
                ce = scb(); V.tensor_tensor(ce[:, :], w3[:, :], t6[:, :], A.is_ge)
                V.tensor_tensor(w3[:, :], w3[:, :], t6[:, :], A.max)  # w36
                cg = scb(); V.tensor_tensor(cg[:, :], w1[:, :], w3[:, :], A.is_ge)
                # Ms0 = ca*cd*cg ; Ms4 = (1-ca)*cd*cg ; Ms2 = cb*(1-cd)*cg ;
                # Ms1 = (1-cb)*(1-cd)*cg ; Ms3 = cf*ce*(1-cg) ; Ms5 = (1-cf)*ce*(1-cg)
                # Ms6 = (1-ce)*(1-cg)
                Ms = [ntb(pp, f"ms{t}") for t in range(7)]
                dg = scb(); V.tensor_tensor(dg[:, :], cd[:, :], cg[:, :], A.mult)
                V.tensor_tensor(Ms[0][:, :], ca[:, :], dg[:, :], A.mult)
                V.tensor_tensor(Ms[4][:, :], dg[:, :], Ms[0][:, :], A.subtract)
                ndg = scb(); V.tensor_tensor(ndg[:, :], cg[:, :], dg[:, :], A.subtract)  # (1-cd)*cg
                V.tensor_tensor(Ms[2][:, :], cb[:, :], ndg[:, :], A.mult)
                V.tensor_tensor(Ms[1][:, :], ndg[:, :], Ms[2][:, :], A.subtract)
                ncg = scb(); AFF(ncg[:, :], cg[:, :], -1.0, 1.0)  # 1-cg
                eg = scb(); V.tensor_tensor(eg[:, :], ce[:, :], ncg[:, :], A.mult)
                V.tensor_tensor(Ms[3][:, :], cf_[:, :], eg[:, :], A.mult)
                V.tensor_tensor(Ms[5][:, :], eg[:, :], Ms[3][:, :], A.subtract)
                V.tensor_tensor(Ms[6][:, :], ncg[:, :], eg[:, :], A.subtract)

                # ---- ranks of |x| (stable) ----
                G01 = scb(); V.tensor_tensor(G01[:, :], yv[0], yv[1], A.is_gt)
                G02 = scb(); V.tensor_tensor(G02[:, :], yv[0], yv[2], A.is_gt)
                G03 = scb(); V.tensor_tensor(G03[:, :], yv[0], yv[3], A.is_gt)
                G12 = scb(); G_CMP(G12[:, :], yv[1], yv[2], A.is_gt)
                G13 = scb(); G_CMP(G13[:, :], yv[1], yv[3], A.is_gt)
                G23 = scb(); V.tensor_tensor(G23[:, :], yv[2], yv[3], A.is_gt)

                r0 = ntb(pp, "r0"); r1 = ntb(pp, "r1"); r2 = ntb(pp, "r2"); r3 = ntb(pp, "r3")
                V.tensor_tensor(r0[:, :], G01[:, :], G02[:, :], A.add)
                V.tensor_tensor(r0[:, :], r0[:, :], G03[:, :], A.add)
                G.tensor_tensor(r1[:, :], G12[:, :], G13[:, :], A.add)
                V.scalar_tensor_tensor(r1[:, :], r1[:, :], 1.0, G01[:, :], A.add, A.subtract)
                V.scalar_tensor_tensor(r2[:, :], G23[:, :], 2.0, G02[:, :], A.add, A.subtract)
                V.tensor_tensor(r2[:, :], r2[:, :], G12[:, :], A.subtract)
                G.tensor_tensor(r3[:, :], G03[:, :], G13[:, :], A.add)
                G.tensor_tensor(r3[:, :], r3[:, :], G23[:, :], A.add)
                AFF(r3[:, :], r3[:, :], -1.0, 3.0)
                rs = [r0, r1, r2, r3]

                # ---- rank masks ----
                e3 = [ntb(pp, f"e3{i}") for i in range(4)]
                z = [ntb(pp, f"z{i}") for i in range(4)]
                eng_z = [V, G, V, G]
                for i in range(4):
                    eng_z[i].tensor_scalar(e3[i][:, :], rs[i][:, :], 3.0, None, A.is_equal)
                    eng_z[(i + 1) % 4].tensor_scalar(z[i][:, :], rs[i][:, :], 0.0, None, A.is_equal)
                e2m = [None, scb(), scb(), scb()]
                for i in (1, 2, 3):
                    G.tensor_scalar(e2m[i][:, :], rs[i][:, :], 2.0, None, A.is_equal)

                p3 = scb(); p2 = scb(); p0 = scb()
                V.scalar_tensor_tensor(p3[:, :], e3[2][:, :], 2.0, e3[1][:, :], A.mult, A.add)
                V.scalar_tensor_tensor(p3[:, :], e3[3][:, :], 3.0, p3[:, :], A.mult, A.add)
                V.scalar_tensor_tensor(p2[:, :], e2m[2][:, :], 2.0, e2m[1][:, :], A.mult, A.add)
                V.scalar_tensor_tensor(p2[:, :], e2m[3][:, :], 3.0, p2[:, :], A.mult, A.add)
                AFF(p0[:, :], z[2][:, :], 2.0)
                G.tensor_tensor(p0[:, :], p0[:, :], z[1][:, :], A.add)
                pz3 = scb(); AFF(pz3[:, :], z[3][:, :], 3.0)
                G.tensor_tensor(p0[:, :], p0[:, :], pz3[:, :], A.add)

                # ---- val2 / val6 ----
                r0hi = scb(); G.tensor_scalar(r0hi[:, :], r0[:, :], 2.0, None, A.is_ge)
                P15 = scb(); V.tensor_tensor(P15[:, :], p2[:, :], p3[:, :], A.add)
                Bv = scb(); AFF(Bv[:, :], P15[:, :], -2.0, 12.0)
                Dv = scb(); AFF(Dv[:, :], P15[:, :], 4.0, -11.0)
                val2 = scb(); V.tensor_tensor(val2[:, :], r0hi[:, :], Dv[:, :], A.mult)
                V.tensor_tensor(val2[:, :], val2[:, :], Bv[:, :], A.add)
                gpc = scb(); V.tensor_tensor(gpc[:, :], p3[:, :], p2[:, :], A.is_gt)
                val6 = scb(); V.tensor_tensor(val6[:, :], p3[:, :], gpc[:, :], A.subtract)
                AFF(val6[:, :], val6[:, :], 4.0, 20.0)
                V.tensor_tensor(val6[:, :], val6[:, :], p2[:, :], A.add)

                # ---- low5 ----
                low = ntb(pp, "low")
                l1 = scb(); V.scalar_tensor_tensor(l1[:, :], p3[:, :], 8.0, Ms[1][:, :], A.add, A.mult)
                l2 = scb(); V.tensor_tensor(l2[:, :], Ms[2][:, :], val2[:, :], A.mult)
                l3 = scb(); AFF(l3[:, :], p0[:, :], 1.0, 12.0)
                G.tensor_tensor(l3[:, :], l3[:, :], Ms[3][:, :], A.mult)
                l5 = scb(); AFF(l5[:, :], p3[:, :], 1.0, 16.0)
                G.tensor_tensor(l5[:, :], l5[:, :], Ms[5][:, :], A.mult)
                V.tensor_tensor(val6[:, :], Ms[6][:, :], val6[:, :], A.mult)
                V.tensor_tensor(low[:, :], Ms[4][:, :], l1[:, :], A.add)
                G.tensor_tensor(l2[:, :], l2[:, :], l3[:, :], A.add)
                V.tensor_tensor(l5[:, :], l5[:, :], val6[:, :], A.add)
                G.tensor_tensor(l2[:, :], l2[:, :], l5[:, :], A.add)
                V.tensor_tensor(low[:, :], low[:, :], l2[:, :], A.add)

                # ---- parity flip ----
                rt = sc()
                G.tensor_tensor(rt[:, :], Ms[1][:, :], Ms[3][:, :], A.add)
                G.tensor_tensor(rt[:, :], rt[:, :], Ms[6][:, :], A.add)
                dl = scb(); V.tensor_tensor(dl[:, :], q[:, :], rt[:, :], A.not_equal)

                # flips: z_i <- dl * z_i ; then vv_i = 1 - 2*flip_i (in place)
                eng_f = [V, G, V, G]
                for i in range(4):
                    eng_f[i].tensor_tensor(z[i][:, :], dl[:, :], z[i][:, :], A.mult)
                    AFF(z[i][:, :], z[i][:, :], -2.0, 1.0)
                # S_i = cc_i * vv_i  (in place onto cc views)
                eng_s = [V, G, V, G]
                for i in range(4):
                    eng_s[i].tensor_tensor(ccv[i], ccv[i], z[i][:, :], A.mult)

                # ---- k = #big coords ; has25 ----
                kacc = scb()
                V.scalar_tensor_tensor(kacc[:, :], Ms[1][:, :], 2.0, Ms[0][:, :], A.mult, A.max)
                for t, kt1 in ((2, 3.0), (3, 4.0), (4, 5.0), (5, 2.0), (6, 3.0)):
                    V.scalar_tensor_tensor(kacc[:, :], Ms[t][:, :], kt1, kacc[:, :], A.mult, A.max)
                has25 = scb(); G.tensor_tensor(has25[:, :], Ms[5][:, :], Ms[6][:, :], A.add)

                # ---- output values ----
                out = wp.tile([P, fd], dt.float32, tag="xt", name="out")  # reuse xt slot
                for i in range(4):
                    u = scb()
                    V.tensor_tensor(u[:, :], rs[i][:, :], kacc[:, :], A.add)
                    V.tensor_scalar(u[:, :], u[:, :], 5.0, None, A.is_ge)
                    b = scb()
                    G.tensor_tensor(b[:, :], e3[i][:, :], has25[:, :], A.mult)
                    V.tensor_tensor(u[:, :], u[:, :], b[:, :], A.add)
                    V.scalar_tensor_tensor(out[:, i::4], u[:, :], 0.5, ccv[i], A.add, A.mult)
                nc.sync.dma_start(vals_d, out[:, :])

                # ---- index bits ----
                pr6 = scb(); V.tensor_tensor(pr6[:, :], ccv[0], ccv[1], A.mult)
                pr5 = scb(); G.tensor_tensor(pr5[:, :], ccv[0], ccv[2], A.mult)
                AFF(pr6[:, :], pr6[:, :], -32.0, 32.0)
                AFF(pr5[:, :], pr5[:, :], -16.0, 16.0)
                q7 = scb(); AFF(q7[:, :], ccv[0], -64.0, 64.0)
                V.tensor_tensor(low[:, :], low[:, :], pr5[:, :], A.add)
                V.tensor_tensor(low[:, :], low[:, :], pr6[:, :], A.add)
                V.tensor_tensor(low[:, :], low[:, :], q7[:, :], A.add)

                idx8 = pp.tile([P, nf], dt.uint8, tag="idx8", name="idx8")
                V.tensor_copy(idx8[:, :], low[:, :])
                nc.sync.dma_start(idx_d, idx8[:, :])

    if fix_waits:
        _fix_waits(nc)
    return nc


_nc_cache = {}


def _get_nc(nf):
    if nf not in _nc_cache:
        _nc_cache[nf] = build_nc(nf)
    return _nc_cache[nf]


def kernel(X, grid=None, grid_norm=None):
    from concourse.bass_utils import run_bass_kernel_spmd

    X = np.ascontiguousarray(np.asarray(X), dtype=np.float32)
    assert X.shape == (N_VECS, 4)
    nf = N_PER_CORE // P
    nc = _get_nc(nf)
    shards = np.split(X, N_CORES, axis=0)
    in_maps = [{"x": s} for s in shards]
    res = run_bass_kernel_spmd(nc, in_maps, list(range(N_CORES)))
    vals = np.concatenate([r["vals"] for r in res.results], axis=0)
    idx = np.concatenate([r["idx"] for r in res.results], axis=0)
    return vals.astype(np.float32), idx.astype(np.uint8)


# revision 19
# speedup vs baseline: 1.1719x; 1.1719x over previous
"""Trainium2 Bass kernel for the D4 codebook (VQ) problem.

Instead of a 256-way argmax, exploits the structure of the codebook: it is
exactly the set of points in the coset (Z+1/2)^4 with even coordinate sum and
norm^2 <= 9.  The nearest codeword is found by scoring 7 magnitude-pattern
types against the sorted |x| values, with a parity-forced sign flip on the
smallest coordinate.  Everything is elementwise f32, no matmul, no gather.

Self-contained: hardcodes shapes (X: [1048576, 4] f32), shards row-blocks
across 8 NeuronCores.
"""

import numpy as np

N_VECS = 1048576
N_CORES = 8
N_PER_CORE = N_VECS // N_CORES  # 131072
P = 128


def _fix_waits(nc):
    """Hoist per-instruction sem waits beyond walrus codegen capacity onto
    standalone EventSemaphore ops inserted just before the instruction.
    Capacities (empirical, this walrus build): EventSemaphore 2, Drain 0,
    2-tensor-input TensorScalarPtr (scalar_tensor_tensor / scan) 0, rest 1."""
    import concourse.mybir as mybir

    def capacity(inst):
        tn = type(inst).__name__
        if tn == "InstEventSemaphore":
            return 2
        if tn == "InstDrain":
            return 0
        if tn == "InstTensorScalarPtr":
            if len(inst.ins or []) >= 2:
                return 0
            return 1
        return 1

    n_fixed = 0
    for f in nc.m.functions:
        for bb in f.blocks:
            out = []
            changed = False
            for inst in bb.instructions:
                si = inst.sync_info
                waits = list(si.on_wait) if si is not None and si.on_wait else []
                cap = capacity(inst)
                if len(waits) > cap:
                    hoist = waits[: len(waits) - cap] if cap else waits
                    keep = waits[len(waits) - cap:] if cap else []
                    for k, w in enumerate(hoist):
                        es = mybir.InstEventSemaphore(
                            name=f"{inst.name}_hw{k}", ins=[], outs=[]
                        )
                        es.engine = inst.engine
                        es.sync_info = mybir.SyncInfo(on_wait=[w], on_update=[])
                        out.append(es)
                        n_fixed += 1
                    inst.sync_info = mybir.SyncInfo(
                        on_wait=keep,
                        on_update=list(si.on_update) if si.on_update else [],
                    )
                    changed = True
                out.append(inst)
            if changed:
                bb.instructions[:] = out
    return n_fixed


def build_nc(nf, fix_waits=True, reps=1, mode="va"):
    """Build the Bass program for one core processing P*nf vectors.

    mode: "va" (default) = DVE + ACT for affines, no GPSIMD (cross-engine
    sync with Pool measured far too costly); "allv" = everything on DVE;
    "mixed" = spread across DVE/GPSIMD/ACT.
    """
    import concourse.bass as bass
    import concourse.mybir as mybir
    import concourse.tile as tile

    dt = mybir.dt
    A = mybir.AluOpType
    Act = mybir.ActivationFunctionType

    n = P * nf
    fd = nf * 4

    nc = bass.Bass()
    x_h = nc.dram_tensor("x", [n, 4], dt.float32, kind="ExternalInput")
    vals_h = nc.dram_tensor("vals", [n, 4], dt.float32, kind="ExternalOutput")
    idx_h = nc.dram_tensor("idx", [n], dt.uint8, kind="ExternalOutput")

    x_d = x_h[:, :].rearrange("(p n) d -> p (n d)", p=P)
    vals_d = vals_h[:, :].rearrange("(p n) d -> p (n d)", p=P)
    idx_d = idx_h[:].rearrange("(p n) -> p n", p=P)

    with tile.TileContext(nc) as tc:
        with (
            tc.tile_pool(name="wide", bufs=1) as wp,
            tc.tile_pool(name="pers", bufs=1) as pp,
            tc.tile_pool(name="scr", bufs=20) as sp,
        ):
            V = nc.vector
            G = nc.gpsimd if mode == "mixed" else nc.vector
            S = nc.scalar
            use_act = mode in ("mixed", "va")
            g_is_pool = mode == "mixed"

            def AFF(out, in_, scale=1.0, bias=0.0):
                """affine: out = in_*scale + bias"""
                if use_act:
                    S.activation(out, in_, Act.Copy, bias=bias, scale=scale)
                else:
                    V.tensor_scalar(out, in_, scale, bias, A.mult, A.add)

            def G_CMP(out, a, b, op):
                """tensor-tensor compare on G (pool lacks TT compares)"""
                if g_is_pool:
                    G.tensor_tensor(out, a, b, A.subtract)
                    G.tensor_scalar(out, out, 0.0, None, op)
                else:
                    G.tensor_tensor(out, a, b, op)

            def nt(pool, tag):
                return pool.tile([P, nf], dt.float32, tag=tag, name=tag)

            def ntb(pool, tag):
                return pool.tile([P, nf], dt.bfloat16, tag=tag + "b", name=tag)

            def sc():
                return nt(sp, "s")

            def scb():
                return ntb(sp, "s")

            for _rep in range(reps):
                # ---- load ----
                xt = wp.tile([P, fd], dt.float32, tag="xt", name="xt")
                nc.sync.dma_start(xt[:, :], x_d)
                xv = [xt[:, i::4] for i in range(4)]

                # ---- wide: y = |x|, cc = sign in {-1,+1} ----
                y = wp.tile([P, fd], dt.float32, tag="y", name="y")
                S.activation(y[:, :], xt[:, :], Act.Abs)
                yv = [y[:, i::4] for i in range(4)]

                cc = wp.tile([P, fd], dt.bfloat16, tag="cc", name="cc")
                G.tensor_scalar(cc[:, :], xt[:, :], 0.0, 2.0, A.is_ge, A.mult)
                AFF(cc[:, :], cc[:, :], 1.0, -1.0)
                ccv = [cc[:, i::4] for i in range(4)]

                # ---- q: parity of #negatives via product sign ----
                p01 = sc(); G.tensor_tensor(p01[:, :], xv[0], xv[1], A.mult)
                p23 = sc(); G.tensor_tensor(p23[:, :], xv[2], xv[3], A.mult)
                G.tensor_tensor(p01[:, :], p01[:, :], p23[:, :], A.mult)
                q = nt(pp, "q"); G.tensor_scalar(q[:, :], p01[:, :], 0.0, None, A.is_lt)

                # ---- sort network: s1 <= . <= s3 <= s4 over |x| ----
                L1 = sc(); V.tensor_tensor(L1[:, :], yv[0], yv[1], A.min)
                H1 = sc(); V.tensor_tensor(H1[:, :], yv[0], yv[1], A.max)
                L2 = sc(); V.tensor_tensor(L2[:, :], yv[2], yv[3], A.min)
                H2 = sc(); V.tensor_tensor(H2[:, :], yv[2], yv[3], A.max)
                s1 = sc(); V.tensor_tensor(s1[:, :], L1[:, :], L2[:, :], A.min)
                mm = sc(); V.tensor_tensor(mm[:, :], L1[:, :], L2[:, :], A.max)
                mh = sc(); V.tensor_tensor(mh[:, :], H1[:, :], H2[:, :], A.min)
                s4 = sc(); V.tensor_tensor(s4[:, :], H1[:, :], H2[:, :], A.max)
                s3 = sc(); V.tensor_tensor(s3[:, :], mm[:, :], mh[:, :], A.max)

                # T = sum |x|
                Ta = sc(); G.tensor_tensor(Ta[:, :], L1[:, :], H1[:, :], A.add)
                Tb = sc(); G.tensor_tensor(Tb[:, :], L2[:, :], H2[:, :], A.add)
                T = sc(); G.tensor_tensor(T[:, :], Ta[:, :], Tb[:, :], A.add)

                # ---- 7 type scores ----
                w12 = sc(); AFF(w12[:, :], s1[:, :], 2.0)
                e2 = sc(); G.tensor_tensor(e2[:, :], w12[:, :], q[:, :], A.mult)
                f2 = sc(); G.tensor_tensor(f2[:, :], w12[:, :], e2[:, :], A.subtract)
                e23 = sc(); AFF(e23[:, :], e2[:, :], 3.0)

                t0 = sc(); V.scalar_tensor_tensor(t0[:, :], T[:, :], -1.0, e2[:, :], A.add, A.subtract)
                t1 = sc(); V.scalar_tensor_tensor(t1[:, :], s4[:, :], 2.0, T[:, :], A.mult, A.add)  # U
                t2 = sc(); V.scalar_tensor_tensor(t2[:, :], s3[:, :], 2.0, t1[:, :], A.mult, A.add)  # U+2s3
                V.scalar_tensor_tensor(t1[:, :], t1[:, :], -3.0, f2[:, :], A.add, A.subtract)
                V.scalar_tensor_tensor(t2[:, :], t2[:, :], -5.0, e2[:, :], A.add, A.subtract)
                t5 = sc(); V.scalar_tensor_tensor(t5[:, :], s4[:, :], 4.0, T[:, :], A.mult, A.add)  # W
                t6 = sc(); V.scalar_tensor_tensor(t6[:, :], s3[:, :], 2.0, t5[:, :], A.mult, A.add)  # W+2s3
                V.scalar_tensor_tensor(t5[:, :], t5[:, :], -7.0, e2[:, :], A.add, A.subtract)
                V.scalar_tensor_tensor(t6[:, :], t6[:, :], -9.0, f2[:, :], A.add, A.subtract)
                t3 = sc(); V.scalar_tensor_tensor(t3[:, :], T[:, :], 3.0, w12[:, :], A.mult, A.subtract)  # 3T-2s1
                V.scalar_tensor_tensor(t3[:, :], t3[:, :], -7.0, f2[:, :], A.add, A.subtract)
                t4 = sc(); AFF(t4[:, :], T[:, :], 3.0)
                V.scalar_tensor_tensor(t4[:, :], t4[:, :], -9.0, e23[:, :], A.add, A.subtract)
                ts_ = [t0, t1, t2, t3, t4, t5, t6]

                # ---- winner masks via comparison tree ----
                # preference order (ties go left): t0 > t4 > t2 > t1 > t3 > t5 > t6
                ca = scb(); V.tensor_tensor(ca[:, :], t0[:, :], t4[:, :], A.is_ge)
                cb = scb(); V.tensor_tensor(cb[:, :], t2[:, :], t1[:, :], A.is_ge)
                cf_ = scb(); V.tensor_tensor(cf_[:, :], t3[:, :], t5[:, :], A.is_ge)
                w1 = sc(); V.tensor_tensor(w1[:, :], t0[:, :], t4[:, :], A.max)
                w2 = sc(); V.tensor_tensor(w2[:, :], t2[:, :], t1[:, :], A.max)
                w3 = sc(); V.tensor_tensor(w3[:, :], t3[:, :], t5[:, :], A.max)
                cd = scb(); V.tensor_tensor(cd[:, :], w1[:, :], w2[:, :], A.is_ge)
                V.tensor_tensor(w1[:, :], w1[:, :], w2[:, :], A.max)  # w12
                ce = scb(); V.tensor_tensor(ce[:, :], w3[:, :], t6[:, :], A.is_ge)
                V.tensor_tensor(w3[:, :], w3[:, :], t6[:, :], A.max)  # w36
                cg = scb(); V.tensor_tensor(cg[:, :], w1[:, :], w3[:, :], A.is_ge)
                # Ms0 = ca*cd*cg ; Ms4 = (1-ca)*cd*cg ; Ms2 = cb*(1-cd)*cg ;
                # Ms1 = (1-cb)*(1-cd)*cg ; Ms3 = cf*ce*(1-cg) ; Ms5 = (1-cf)*ce*(1-cg)
                # Ms6 = (1-ce)*(1-cg)
                Ms = [ntb(pp, f"ms{t}") for t in range(7)]
                dg = scb(); V.tensor_tensor(dg[:, :], cd[:, :], cg[:, :], A.mult)
                V.tensor_tensor(Ms[0][:, :], ca[:, :], dg[:, :], A.mult)
                V.tensor_tensor(Ms[4][:, :], dg[:, :], Ms[0][:, :], A.subtract)
                ndg = scb(); V.tensor_tensor(ndg[:, :], cg[:, :], dg[:, :], A.subtract)  # (1-cd)*cg
                V.tensor_tensor(Ms[2][:, :], cb[:, :], ndg[:, :], A.mult)
                V.tensor_tensor(Ms[1][:, :], ndg[:, :], Ms[2][:, :], A.subtract)
                ncg = scb(); AFF(ncg[:, :], cg[:, :], -1.0, 1.0)  # 1-cg
                eg = scb(); V.tensor_tensor(eg[:, :], ce[:, :], ncg[:, :], A.mult)
                V.tensor_tensor(Ms[3][:, :], cf_[:, :], eg[:, :], A.mult)
                V.tensor_tensor(Ms[5][:, :], eg[:, :], Ms[3][:, :], A.subtract)
                V.tensor_tensor(Ms[6][:, :], ncg[:, :], eg[:, :], A.subtract)

                # ---- ranks of |x| (stable) ----
                G01 = scb(); V.tensor_tensor(G01[:, :], yv[0], yv[1], A.is_gt)
                G02 = scb(); V.tensor_tensor(G02[:, :], yv[0], yv[2], A.is_gt)
                G03 = scb(); V.tensor_tensor(G03[:, :], yv[0], yv[3], A.is_gt)
                G12 = scb(); G_CMP(G12[:, :], yv[1], yv[2], A.is_gt)
                G13 = scb(); G_CMP(G13[:, :], yv[1], yv[3], A.is_gt)
                G23 = scb(); V.tensor_tensor(G23[:, :], yv[2], yv[3], A.is_gt)

                r0 = ntb(pp, "r0"); r1 = ntb(pp, "r1"); r2 = ntb(pp, "r2"); r3 = ntb(pp, "r3")
                V.tensor_tensor(r0[:, :], G01[:, :], G02[:, :], A.add)
                V.tensor_tensor(r0[:, :], r0[:, :], G03[:, :], A.add)
                G.tensor_tensor(r1[:, :], G12[:, :], G13[:, :], A.add)
                V.scalar_tensor_tensor(r1[:, :], r1[:, :], 1.0, G01[:, :], A.add, A.subtract)
                V.scalar_tensor_tensor(r2[:, :], G23[:, :], 2.0, G02[:, :], A.add, A.subtract)
                V.tensor_tensor(r2[:, :], r2[:, :], G12[:, :], A.subtract)
                G.tensor_tensor(r3[:, :], G03[:, :], G13[:, :], A.add)
                G.tensor_tensor(r3[:, :], r3[:, :], G23[:, :], A.add)
                AFF(r3[:, :], r3[:, :], -1.0, 3.0)
                rs = [r0, r1, r2, r3]

                # ---- rank masks ----
                e3 = [ntb(pp, f"e3{i}") for i in range(4)]
                z = [ntb(pp, f"z{i}") for i in range(4)]
                eng_z = [V, G, V, G]
                for i in range(4):
                    eng_z[i].tensor_scalar(e3[i][:, :], rs[i][:, :], 3.0, None, A.is_equal)
                    eng_z[(i + 1) % 4].tensor_scalar(z[i][:, :], rs[i][:, :], 0.0, None, A.is_equal)
                e2m = [None, scb(), scb(), scb()]
                for i in (1, 2, 3):
                    G.tensor_scalar(e2m[i][:, :], rs[i][:, :], 2.0, None, A.is_equal)

                p3 = scb(); p2 = scb(); p0 = scb()
                V.scalar_tensor_tensor(p3[:, :], e3[2][:, :], 2.0, e3[1][:, :], A.mult, A.add)
                V.scalar_tensor_tensor(p3[:, :], e3[3][:, :], 3.0, p3[:, :], A.mult, A.add)
                V.scalar_tensor_tensor(p2[:, :], e2m[2][:, :], 2.0, e2m[1][:, :], A.mult, A.add)
                V.scalar_tensor_tensor(p2[:, :], e2m[3][:, :], 3.0, p2[:, :], A.mult, A.add)
                AFF(p0[:, :], z[2][:, :], 2.0)
                G.tensor_tensor(p0[:, :], p0[:, :], z[1][:, :], A.add)
                pz3 = scb(); AFF(pz3[:, :], z[3][:, :], 3.0)
                G.tensor_tensor(p0[:, :], p0[:, :], pz3[:, :], A.add)

                # ---- val2 / val6 ----
                r0hi = scb(); G.tensor_scalar(r0hi[:, :], r0[:, :], 2.0, None, A.is_ge)
                P15 = scb(); V.tensor_tensor(P15[:, :], p2[:, :], p3[:, :], A.add)
                Bv = scb(); AFF(Bv[:, :], P15[:, :], -2.0, 12.0)
                Dv = scb(); AFF(Dv[:, :], P15[:, :], 4.0, -11.0)
                val2 = scb(); V.tensor_tensor(val2[:, :], r0hi[:, :], Dv[:, :], A.mult)
                V.tensor_tensor(val2[:, :], val2[:, :], Bv[:, :], A.add)
                gpc = scb(); V.tensor_tensor(gpc[:, :], p3[:, :], p2[:, :], A.is_gt)
                val6 = scb(); V.tensor_tensor(val6[:, :], p3[:, :], gpc[:, :], A.subtract)
                AFF(val6[:, :], val6[:, :], 4.0, 20.0)
                V.tensor_tensor(val6[:, :], val6[:, :], p2[:, :], A.add)

                # ---- low5 ----
                low = ntb(pp, "low")
                l1 = scb(); V.scalar_tensor_tensor(l1[:, :], p3[:, :], 8.0, Ms[1][:, :], A.add, A.mult)
                l2 = scb(); V.tensor_tensor(l2[:, :], Ms[2][:, :], val2[:, :], A.mult)
                l3 = scb(); AFF(l3[:, :], p0[:, :], 1.0, 12.0)
                G.tensor_tensor(l3[:, :], l3[:, :], Ms[3][:, :], A.mult)
                l5 = scb(); AFF(l5[:, :], p3[:, :], 1.0, 16.0)
                G.tensor_tensor(l5[:, :], l5[:, :], Ms[5][:, :], A.mult)
                V.tensor_tensor(val6[:, :], Ms[6][:, :], val6[:, :], A.mult)
                V.tensor_tensor(low[:, :], Ms[4][:, :], l1[:, :], A.add)
                G.tensor_tensor(l2[:, :], l2[:, :], l3[:, :], A.add)
                V.tensor_tensor(l5[:, :], l5[:, :], val6[:, :], A.add)
                G.tensor_tensor(l2[:, :], l2[:, :], l5[:, :], A.add)
                V.tensor_tensor(low[:, :], low[:, :], l2[:, :], A.add)

                # ---- parity flip ----
                rt = sc()
                G.tensor_tensor(rt[:, :], Ms[1][:, :], Ms[3][:, :], A.add)
                G.tensor_tensor(rt[:, :], rt[:, :], Ms[6][:, :], A.add)
                dl = scb(); V.tensor_tensor(dl[:, :], q[:, :], rt[:, :], A.not_equal)

                # flips: z_i <- dl * z_i ; then vv_i = 1 - 2*flip_i (in place)
                eng_f = [V, G, V, G]
                for i in range(4):
                    eng_f[i].tensor_tensor(z[i][:, :], dl[:, :], z[i][:, :], A.mult)
                    AFF(z[i][:, :], z[i][:, :], -2.0, 1.0)
                # S_i = cc_i * vv_i  (in place onto cc views)
                eng_s = [V, G, V, G]
                for i in range(4):
                    eng_s[i].tensor_tensor(ccv[i], ccv[i], z[i][:, :], A.mult)

                # ---- k = #big coords ; has25 ----
                kacc = scb()
                V.scalar_tensor_tensor(kacc[:, :], Ms[1][:, :], 2.0, Ms[0][:, :], A.mult, A.max)
                for t, kt1 in ((2, 3.0), (3, 4.0), (4, 5.0), (5, 2.0), (6, 3.0)):
                    V.scalar_tensor_tensor(kacc[:, :], Ms[t][:, :], kt1, kacc[:, :], A.mult, A.max)
                has25 = scb(); G.tensor_tensor(has25[:, :], Ms[5][:, :], Ms[6][:, :], A.add)

                # ---- output values ----
                out = wp.tile([P, fd], dt.float32, tag="xt", name="out")  # reuse xt slot
                for i in range(4):
                    u = scb()
                    V.tensor_tensor(u[:, :], rs[i][:, :], kacc[:, :], A.add)
                    V.tensor_scalar(u[:, :], u[:, :], 5.0, None, A.is_ge)
                    b = scb()
                    G.tensor_tensor(b[:, :], e3[i][:, :], has25[:, :], A.mult)
                    V.tensor_tensor(u[:, :], u[:, :], b[:, :], A.add)
                    V.scalar_tensor_tensor(out[:, i::4], u[:, :], 0.5, ccv[i], A.add, A.mult)
                nc.sync.dma_start(vals_d, out[:, :])

                # ---- index bits ----
                pr6 = scb(); V.tensor_tensor(pr6[:, :], ccv[0], ccv[1], A.mult)
                pr5 = scb(); G.tensor_tensor(pr5[:, :], ccv[0], ccv[2], A.mult)
                AFF(pr6[:, :], pr6[:, :], -32.0, 32.0)
                AFF(pr5[:, :], pr5[:, :], -16.0, 16.0)
                q7 = scb(); AFF(q7[:, :], ccv[0], -64.0, 64.0)
                V.tensor_tensor(low[:, :], low[:, :], pr5[:, :], A.add)
                V.tensor_tensor(low[:, :], low[:, :], pr6[:, :], A.add)
                V.tensor_tensor(low[:, :], low[:, :], q7[:, :], A.add)

                idx8 = pp.tile([P, nf], dt.uint8, tag="idx8", name="idx8")
                V.tensor_copy(idx8[:, :], low[:, :])
                nc.sync.dma_start(idx_d, idx8[:, :])

    if fix_waits:
        _fix_waits(nc)
    return nc


_nc_cache = {}


def _get_nc(nf):
    if nf not in _nc_cache:
        _nc_cache[nf] = build_nc(nf)
    return _nc_cache[nf]


def kernel(X, grid=None, grid_norm=None):
    from concourse.bass_utils import run_bass_kernel_spmd

    X = np.ascontiguousarray(np.asarray(X), dtype=np.float32)
    assert X.shape == (N_VECS, 4)
    nf = N_PER_CORE // P
    nc = _get_nc(nf)
    shards = np.split(X, N_CORES, axis=0)
    in_maps = [{"x": s} for s in shards]
    res = run_bass_kernel_spmd(nc, in_maps, list(range(N_CORES)))
    vals = np.concatenate([r["vals"] for r in res.results], axis=0)
    idx = np.concatenate([r["idx"] for r in res.results], axis=0)
    return vals.astype(np.float32), idx.astype(np.uint8)


# revision 20
# speedup vs baseline: 1.3443x; 1.1471x over previous
"""Trainium2 Bass kernel for the D4 codebook (VQ) problem.

Instead of a 256-way argmax, exploits the structure of the codebook: it is
exactly the set of points in the coset (Z+1/2)^4 with even coordinate sum and
norm^2 <= 9.  The nearest codeword is found by scoring 7 magnitude-pattern
types against the sorted |x| values, with a parity-forced sign flip on the
smallest coordinate.  Everything is elementwise f32, no matmul, no gather.

Self-contained: hardcodes shapes (X: [1048576, 4] f32), shards row-blocks
across 8 NeuronCores.
"""

import numpy as np

N_VECS = 1048576
N_CORES = 8
N_PER_CORE = N_VECS // N_CORES  # 131072
P = 128


def _fix_waits(nc):
    """Hoist per-instruction sem waits beyond walrus codegen capacity onto
    standalone EventSemaphore ops inserted just before the instruction.
    Capacities (empirical, this walrus build): EventSemaphore 2, Drain 0,
    2-tensor-input TensorScalarPtr (scalar_tensor_tensor / scan) 0, rest 1."""
    import concourse.mybir as mybir

    def capacity(inst):
        tn = type(inst).__name__
        if tn == "InstEventSemaphore":
            return 2
        if tn == "InstDrain":
            return 0
        if tn == "InstTensorScalarPtr":
            if len(inst.ins or []) >= 2:
                return 0
            return 1
        return 1

    n_fixed = 0
    for f in nc.m.functions:
        for bb in f.blocks:
            out = []
            changed = False
            for inst in bb.instructions:
                si = inst.sync_info
                waits = list(si.on_wait) if si is not None and si.on_wait else []
                cap = capacity(inst)
                if len(waits) > cap:
                    hoist = waits[: len(waits) - cap] if cap else waits
                    keep = waits[len(waits) - cap:] if cap else []
                    for k, w in enumerate(hoist):
                        es = mybir.InstEventSemaphore(
                            name=f"{inst.name}_hw{k}", ins=[], outs=[]
                        )
                        es.engine = inst.engine
                        es.sync_info = mybir.SyncInfo(on_wait=[w], on_update=[])
                        out.append(es)
                        n_fixed += 1
                    inst.sync_info = mybir.SyncInfo(
                        on_wait=keep,
                        on_update=list(si.on_update) if si.on_update else [],
                    )
                    changed = True
                out.append(inst)
            if changed:
                bb.instructions[:] = out
    return n_fixed


def build_nc(nf, fix_waits=True, reps=1, mode="va"):
    """Build the Bass program for one core processing P*nf vectors.

    mode: "va" (default) = DVE + ACT for affines, no GPSIMD (cross-engine
    sync with Pool measured far too costly); "allv" = everything on DVE;
    "mixed" = spread across DVE/GPSIMD/ACT.
    """
    import concourse.bass as bass
    import concourse.mybir as mybir
    import concourse.tile as tile

    dt = mybir.dt
    A = mybir.AluOpType
    Act = mybir.ActivationFunctionType

    n = P * nf
    fd = nf * 4

    nc = bass.Bass()
    x_h = nc.dram_tensor("x", [n, 4], dt.float32, kind="ExternalInput")
    vals_h = nc.dram_tensor("vals", [n, 4], dt.float32, kind="ExternalOutput")
    idx_h = nc.dram_tensor("idx", [n], dt.uint8, kind="ExternalOutput")

    x_d = x_h[:, :].rearrange("(p n) d -> p (n d)", p=P)
    vals_d = vals_h[:, :].rearrange("(p n) d -> p (n d)", p=P)
    idx_d = idx_h[:].rearrange("(p n) -> p n", p=P)

    with tile.TileContext(nc) as tc:
        with (
            tc.tile_pool(name="wide", bufs=1) as wp,
            tc.tile_pool(name="pers", bufs=1) as pp,
            tc.tile_pool(name="scr", bufs=20) as sp,
        ):
            V = nc.vector
            G = nc.gpsimd if mode == "mixed" else nc.vector
            S = nc.scalar
            use_act = mode in ("mixed", "va")
            g_is_pool = mode == "mixed"

            def VAFF(out, in_, scale=1.0, bias=0.0):
                V.tensor_scalar(out, in_, scale, bias, A.mult, A.add)

            def AFF(out, in_, scale=1.0, bias=0.0):
                """affine: out = in_*scale + bias"""
                if use_act:
                    S.activation(out, in_, Act.Copy, bias=bias, scale=scale)
                else:
                    V.tensor_scalar(out, in_, scale, bias, A.mult, A.add)

            def G_CMP(out, a, b, op):
                """tensor-tensor compare on G (pool lacks TT compares)"""
                if g_is_pool:
                    G.tensor_tensor(out, a, b, A.subtract)
                    G.tensor_scalar(out, out, 0.0, None, op)
                else:
                    G.tensor_tensor(out, a, b, op)

            def nt(pool, tag):
                return pool.tile([P, nf], dt.float32, tag=tag, name=tag)

            def ntb(pool, tag):
                return pool.tile([P, nf], dt.bfloat16, tag=tag + "b", name=tag)

            def sc():
                return nt(sp, "s")

            def scb():
                return ntb(sp, "s")

            for _rep in range(reps):
                # ---- load ----
                xt = wp.tile([P, fd], dt.float32, tag="xt", name="xt")
                nc.sync.dma_start(xt[:, :], x_d)
                xv = [xt[:, i::4] for i in range(4)]

                # ---- wide: y = |x|, cc = sign in {-1,+1} ----
                y = wp.tile([P, fd], dt.float32, tag="y", name="y")
                S.activation(y[:, :], xt[:, :], Act.Abs)
                yv = [y[:, i::4] for i in range(4)]

                cc = wp.tile([P, fd], dt.bfloat16, tag="cc", name="cc")
                G.tensor_scalar(cc[:, :], xt[:, :], 0.0, 2.0, A.is_ge, A.mult)
                AFF(cc[:, :], cc[:, :], 1.0, -1.0)
                ccv = [cc[:, i::4] for i in range(4)]

                # ---- q: parity of #negatives via product sign ----
                p01 = sc(); G.tensor_tensor(p01[:, :], xv[0], xv[1], A.mult)
                p23 = sc(); G.tensor_tensor(p23[:, :], xv[2], xv[3], A.mult)
                G.tensor_tensor(p01[:, :], p01[:, :], p23[:, :], A.mult)
                q = nt(pp, "q"); G.tensor_scalar(q[:, :], p01[:, :], 0.0, None, A.is_lt)

                # ---- sort network: s1 <= . <= s3 <= s4 over |x| ----
                L1 = sc(); V.tensor_tensor(L1[:, :], yv[0], yv[1], A.min)
                H1 = sc(); V.tensor_tensor(H1[:, :], yv[0], yv[1], A.max)
                L2 = sc(); V.tensor_tensor(L2[:, :], yv[2], yv[3], A.min)
                H2 = sc(); V.tensor_tensor(H2[:, :], yv[2], yv[3], A.max)
                s1 = sc(); V.tensor_tensor(s1[:, :], L1[:, :], L2[:, :], A.min)
                mm = sc(); V.tensor_tensor(mm[:, :], L1[:, :], L2[:, :], A.max)
                mh = sc(); V.tensor_tensor(mh[:, :], H1[:, :], H2[:, :], A.min)
                s4 = sc(); V.tensor_tensor(s4[:, :], H1[:, :], H2[:, :], A.max)
                s3 = sc(); V.tensor_tensor(s3[:, :], mm[:, :], mh[:, :], A.max)

                # T = sum |x|
                Ta = sc(); G.tensor_tensor(Ta[:, :], L1[:, :], H1[:, :], A.add)
                Tb = sc(); G.tensor_tensor(Tb[:, :], L2[:, :], H2[:, :], A.add)
                T = sc(); G.tensor_tensor(T[:, :], Ta[:, :], Tb[:, :], A.add)

                # ---- 7 type scores ----
                w12 = sc(); VAFF(w12[:, :], s1[:, :], 2.0)
                e2 = sc(); G.tensor_tensor(e2[:, :], w12[:, :], q[:, :], A.mult)
                f2 = sc(); G.tensor_tensor(f2[:, :], w12[:, :], e2[:, :], A.subtract)
                e23 = sc(); VAFF(e23[:, :], e2[:, :], 3.0)

                t0 = sc(); V.scalar_tensor_tensor(t0[:, :], T[:, :], -1.0, e2[:, :], A.add, A.subtract)
                t1 = sc(); V.scalar_tensor_tensor(t1[:, :], s4[:, :], 2.0, T[:, :], A.mult, A.add)  # U
                t2 = sc(); V.scalar_tensor_tensor(t2[:, :], s3[:, :], 2.0, t1[:, :], A.mult, A.add)  # U+2s3
                V.scalar_tensor_tensor(t1[:, :], t1[:, :], -3.0, f2[:, :], A.add, A.subtract)
                V.scalar_tensor_tensor(t2[:, :], t2[:, :], -5.0, e2[:, :], A.add, A.subtract)
                t5 = sc(); V.scalar_tensor_tensor(t5[:, :], s4[:, :], 4.0, T[:, :], A.mult, A.add)  # W
                t6 = sc(); V.scalar_tensor_tensor(t6[:, :], s3[:, :], 2.0, t5[:, :], A.mult, A.add)  # W+2s3
                V.scalar_tensor_tensor(t5[:, :], t5[:, :], -7.0, e2[:, :], A.add, A.subtract)
                V.scalar_tensor_tensor(t6[:, :], t6[:, :], -9.0, f2[:, :], A.add, A.subtract)
                t3 = sc(); V.scalar_tensor_tensor(t3[:, :], T[:, :], 3.0, w12[:, :], A.mult, A.subtract)  # 3T-2s1
                V.scalar_tensor_tensor(t3[:, :], t3[:, :], -7.0, f2[:, :], A.add, A.subtract)
                t4 = sc(); VAFF(t4[:, :], T[:, :], 3.0)
                V.scalar_tensor_tensor(t4[:, :], t4[:, :], -9.0, e23[:, :], A.add, A.subtract)
                ts_ = [t0, t1, t2, t3, t4, t5, t6]

                # ---- winner masks via comparison tree ----
                # preference order (ties go left): t0 > t4 > t2 > t1 > t3 > t5 > t6
                ca = scb(); V.tensor_tensor(ca[:, :], t0[:, :], t4[:, :], A.is_ge)
                cb = scb(); V.tensor_tensor(cb[:, :], t2[:, :], t1[:, :], A.is_ge)
                cf_ = scb(); V.tensor_tensor(cf_[:, :], t3[:, :], t5[:, :], A.is_ge)
                w1 = sc(); V.tensor_tensor(w1[:, :], t0[:, :], t4[:, :], A.max)
                w2 = sc(); V.tensor_tensor(w2[:, :], t2[:, :], t1[:, :], A.max)
                w3 = sc(); V.tensor_tensor(w3[:, :], t3[:, :], t5[:, :], A.max)
                cd = scb(); V.tensor_tensor(cd[:, :], w1[:, :], w2[:, :], A.is_ge)
                V.tensor_tensor(w1[:, :], w1[:, :], w2[:, :], A.max)  # w12
                ce = scb(); V.tensor_tensor(ce[:, :], w3[:, :], t6[:, :], A.is_ge)
                V.tensor_tensor(w3[:, :], w3[:, :], t6[:, :], A.max)  # w36
                cg = scb(); V.tensor_tensor(cg[:, :], w1[:, :], w3[:, :], A.is_ge)
                # Ms0 = ca*cd*cg ; Ms4 = (1-ca)*cd*cg ; Ms2 = cb*(1-cd)*cg ;
                # Ms1 = (1-cb)*(1-cd)*cg ; Ms3 = cf*ce*(1-cg) ; Ms5 = (1-cf)*ce*(1-cg)
                # Ms6 = (1-ce)*(1-cg)
                Ms = [ntb(pp, f"ms{t}") for t in range(7)]
                dg = scb(); V.tensor_tensor(dg[:, :], cd[:, :], cg[:, :], A.mult)
                V.tensor_tensor(Ms[0][:, :], ca[:, :], dg[:, :], A.mult)
                V.tensor_tensor(Ms[4][:, :], dg[:, :], Ms[0][:, :], A.subtract)
                ndg = scb(); V.tensor_tensor(ndg[:, :], cg[:, :], dg[:, :], A.subtract)  # (1-cd)*cg
                V.tensor_tensor(Ms[2][:, :], cb[:, :], ndg[:, :], A.mult)
                V.tensor_tensor(Ms[1][:, :], ndg[:, :], Ms[2][:, :], A.subtract)
                ncg = scb(); VAFF(ncg[:, :], cg[:, :], -1.0, 1.0)  # 1-cg
                eg = scb(); V.tensor_tensor(eg[:, :], ce[:, :], ncg[:, :], A.mult)
                V.tensor_tensor(Ms[3][:, :], cf_[:, :], eg[:, :], A.mult)
                V.tensor_tensor(Ms[5][:, :], eg[:, :], Ms[3][:, :], A.subtract)
                V.tensor_tensor(Ms[6][:, :], ncg[:, :], eg[:, :], A.subtract)

                # ---- ranks of |x| (stable) ----
                G01 = scb(); V.tensor_tensor(G01[:, :], yv[0], yv[1], A.is_gt)
                G02 = scb(); V.tensor_tensor(G02[:, :], yv[0], yv[2], A.is_gt)
                G03 = scb(); V.tensor_tensor(G03[:, :], yv[0], yv[3], A.is_gt)
                G12 = scb(); G_CMP(G12[:, :], yv[1], yv[2], A.is_gt)
                G13 = scb(); G_CMP(G13[:, :], yv[1], yv[3], A.is_gt)
                G23 = scb(); V.tensor_tensor(G23[:, :], yv[2], yv[3], A.is_gt)

                r0 = ntb(pp, "r0"); r1 = ntb(pp, "r1"); r2 = ntb(pp, "r2"); r3 = ntb(pp, "r3")
                V.tensor_tensor(r0[:, :], G01[:, :], G02[:, :], A.add)
                V.tensor_tensor(r0[:, :], r0[:, :], G03[:, :], A.add)
                G.tensor_tensor(r1[:, :], G12[:, :], G13[:, :], A.add)
                V.scalar_tensor_tensor(r1[:, :], r1[:, :], 1.0, G01[:, :], A.add, A.subtract)
                V.scalar_tensor_tensor(r2[:, :], G23[:, :], 2.0, G02[:, :], A.add, A.subtract)
                V.tensor_tensor(r2[:, :], r2[:, :], G12[:, :], A.subtract)
                G.tensor_tensor(r3[:, :], G03[:, :], G13[:, :], A.add)
                G.tensor_tensor(r3[:, :], r3[:, :], G23[:, :], A.add)
                AFF(r3[:, :], r3[:, :], -1.0, 3.0)
                rs = [r0, r1, r2, r3]

                # ---- rank masks ----
                e3 = [ntb(pp, f"e3{i}") for i in range(4)]
                z = [ntb(pp, f"z{i}") for i in range(4)]
                eng_z = [V, G, V, G]
                for i in range(4):
                    eng_z[i].tensor_scalar(e3[i][:, :], rs[i][:, :], 3.0, None, A.is_equal)
                    eng_z[(i + 1) % 4].tensor_scalar(z[i][:, :], rs[i][:, :], 0.0, None, A.is_equal)
                e2m = [None, scb(), scb(), scb()]
                for i in (1, 2, 3):
                    G.tensor_scalar(e2m[i][:, :], rs[i][:, :], 2.0, None, A.is_equal)

                p3 = scb(); p2 = scb(); p0 = scb()
                V.scalar_tensor_tensor(p3[:, :], e3[2][:, :], 2.0, e3[1][:, :], A.mult, A.add)
                V.scalar_tensor_tensor(p3[:, :], e3[3][:, :], 3.0, p3[:, :], A.mult, A.add)
                V.scalar_tensor_tensor(p2[:, :], e2m[2][:, :], 2.0, e2m[1][:, :], A.mult, A.add)
                V.scalar_tensor_tensor(p2[:, :], e2m[3][:, :], 3.0, p2[:, :], A.mult, A.add)
                AFF(p0[:, :], z[2][:, :], 2.0)
                G.tensor_tensor(p0[:, :], p0[:, :], z[1][:, :], A.add)
                pz3 = scb(); AFF(pz3[:, :], z[3][:, :], 3.0)
                G.tensor_tensor(p0[:, :], p0[:, :], pz3[:, :], A.add)

                # ---- val2 / val6 ----
                r0hi = scb(); G.tensor_scalar(r0hi[:, :], r0[:, :], 2.0, None, A.is_ge)
                P15 = scb(); V.tensor_tensor(P15[:, :], p2[:, :], p3[:, :], A.add)
                Bv = scb(); AFF(Bv[:, :], P15[:, :], -2.0, 12.0)
                Dv = scb(); AFF(Dv[:, :], P15[:, :], 4.0, -11.0)
                val2 = scb(); V.tensor_tensor(val2[:, :], r0hi[:, :], Dv[:, :], A.mult)
                V.tensor_tensor(val2[:, :], val2[:, :], Bv[:, :], A.add)
                gpc = scb(); V.tensor_tensor(gpc[:, :], p3[:, :], p2[:, :], A.is_gt)
                val6 = scb(); V.tensor_tensor(val6[:, :], p3[:, :], gpc[:, :], A.subtract)
                AFF(val6[:, :], val6[:, :], 4.0, 20.0)
                V.tensor_tensor(val6[:, :], val6[:, :], p2[:, :], A.add)

                # ---- low5 ----
                low = ntb(pp, "low")
                l1 = scb(); V.scalar_tensor_tensor(l1[:, :], p3[:, :], 8.0, Ms[1][:, :], A.add, A.mult)
                l2 = scb(); V.tensor_tensor(l2[:, :], Ms[2][:, :], val2[:, :], A.mult)
                l3 = scb(); AFF(l3[:, :], p0[:, :], 1.0, 12.0)
                G.tensor_tensor(l3[:, :], l3[:, :], Ms[3][:, :], A.mult)
                l5 = scb(); AFF(l5[:, :], p3[:, :], 1.0, 16.0)
                G.tensor_tensor(l5[:, :], l5[:, :], Ms[5][:, :], A.mult)
                V.tensor_tensor(val6[:, :], Ms[6][:, :], val6[:, :], A.mult)
                V.tensor_tensor(low[:, :], Ms[4][:, :], l1[:, :], A.add)
                G.tensor_tensor(l2[:, :], l2[:, :], l3[:, :], A.add)
                V.tensor_tensor(l5[:, :], l5[:, :], val6[:, :], A.add)
                G.tensor_tensor(l2[:, :], l2[:, :], l5[:, :], A.add)
                V.tensor_tensor(low[:, :], low[:, :], l2[:, :], A.add)

                # ---- parity flip ----
                rt = sc()
                G.tensor_tensor(rt[:, :], Ms[1][:, :], Ms[3][:, :], A.add)
                G.tensor_tensor(rt[:, :], rt[:, :], Ms[6][:, :], A.add)
                dl = scb(); V.tensor_tensor(dl[:, :], q[:, :], rt[:, :], A.not_equal)

                # flips: z_i <- dl * z_i ; then vv_i = 1 - 2*flip_i (in place)
                eng_f = [V, G, V, G]
                for i in range(4):
                    eng_f[i].tensor_tensor(z[i][:, :], dl[:, :], z[i][:, :], A.mult)
                    VAFF(z[i][:, :], z[i][:, :], -2.0, 1.0)
                # S_i = cc_i * vv_i  (in place onto cc views)
                eng_s = [V, G, V, G]
                for i in range(4):
                    eng_s[i].tensor_tensor(ccv[i], ccv[i], z[i][:, :], A.mult)

                # ---- k = #big coords ; has25 ----
                kacc = scb()
                V.scalar_tensor_tensor(kacc[:, :], Ms[1][:, :], 2.0, Ms[0][:, :], A.mult, A.max)
                for t, kt1 in ((2, 3.0), (3, 4.0), (4, 5.0), (5, 2.0), (6, 3.0)):
                    V.scalar_tensor_tensor(kacc[:, :], Ms[t][:, :], kt1, kacc[:, :], A.mult, A.max)
                has25 = scb(); G.tensor_tensor(has25[:, :], Ms[5][:, :], Ms[6][:, :], A.add)

                # ---- output values ----
                out = wp.tile([P, fd], dt.float32, tag="xt", name="out")  # reuse xt slot
                for i in range(4):
                    u = scb()
                    V.tensor_tensor(u[:, :], rs[i][:, :], kacc[:, :], A.add)
                    V.tensor_scalar(u[:, :], u[:, :], 5.0, None, A.is_ge)
                    b = scb()
                    G.tensor_tensor(b[:, :], e3[i][:, :], has25[:, :], A.mult)
                    V.tensor_tensor(u[:, :], u[:, :], b[:, :], A.add)
                    V.scalar_tensor_tensor(out[:, i::4], u[:, :], 0.5, ccv[i], A.add, A.mult)
                nc.sync.dma_start(vals_d, out[:, :])

                # ---- index bits ----
                pr6 = scb(); V.tensor_tensor(pr6[:, :], ccv[0], ccv[1], A.mult)
                pr5 = scb(); G.tensor_tensor(pr5[:, :], ccv[0], ccv[2], A.mult)
                AFF(pr6[:, :], pr6[:, :], -32.0, 32.0)
                AFF(pr5[:, :], pr5[:, :], -16.0, 16.0)
                q7 = scb(); AFF(q7[:, :], ccv[0], -64.0, 64.0)
                V.tensor_tensor(low[:, :], low[:, :], pr5[:, :], A.add)
                V.tensor_tensor(low[:, :], low[:, :], pr6[:, :], A.add)
                V.tensor_tensor(low[:, :], low[:, :], q7[:, :], A.add)

                idx8 = pp.tile([P, nf], dt.uint8, tag="idx8", name="idx8")
                V.tensor_copy(idx8[:, :], low[:, :])
                nc.sync.dma_start(idx_d, idx8[:, :])

    if fix_waits:
        _fix_waits(nc)
    return nc


_nc_cache = {}


def _get_nc(nf):
    if nf not in _nc_cache:
        _nc_cache[nf] = build_nc(nf)
    return _nc_cache[nf]


def kernel(X, grid=None, grid_norm=None):
    from concourse.bass_utils import run_bass_kernel_spmd

    X = np.ascontiguousarray(np.asarray(X), dtype=np.float32)
    assert X.shape == (N_VECS, 4)
    nf = N_PER_CORE // P
    nc = _get_nc(nf)
    shards = np.split(X, N_CORES, axis=0)
    in_maps = [{"x": s} for s in shards]
    res = run_bass_kernel_spmd(nc, in_maps, list(range(N_CORES)))
    vals = np.concatenate([r["vals"] for r in res.results], axis=0)
    idx = np.concatenate([r["idx"] for r in res.results], axis=0)
    return vals.astype(np.float32), idx.astype(np.uint8)
